# revision 12
# baseline (speedup 1.0000x reference)
"""Compressed multi-head attention (H=1) TRN2 Bass kernel, v3.

Reference computation (B=4, S=4096, E=D=1024, H=1, CF=4, Sc=1024):
    qkv = x @ w_qkv.T + b_qkv ; q,k,v = split(qkv)
    kc  = conv1d_stride4(k) + bk ; vc = conv1d_stride4(v) + bv      # [B,Sc,D]
    scores = q @ kc.T / sqrt(D)   (+ causal tril(S,Sc) mask)
    attn = softmax(scores); out = attn @ vc
    y = out @ w_out.T + b_out                                        # [B,S,D]

Sharding: 8 cores = 4 batches x 2 token-halves.  Each core compresses
k/v for ITS half of the batch tokens only (512 compressed tokens).

Device algebra (v3, all operands bf16, PSUM fp32):
  - k/v projections are FOLDED into the compress conv on the host
    (one GEMM per path, contraction CF*E = 4096 over de-interleaved x).
  - the out-projection is pre-folded into the v weights on the host.
  - THE Q PROJECTION IS FOLDED INTO kc: scores = q @ kc.T =
    x @ (wq.T s) @ kc.T = x @ M.T with M.T = (wq s).T @ kc.T computed
    ON DEVICE as a small GEMM over the core's local 512 kc columns
    (64 matmuls instead of the 256 a full q projection needs).  The
    M.T halves are exchanged with the pair partner (1MB bf16
    AllGather) -- this also replaces the kc exchange entirely.
  - phase order K -> M -> V -> D: the M AllGather overlaps phase V,
    the vc AllGather overlaps the first two score groups of phase D.
  - softmax is unnormalized on device: exp(scores) (no max needed,
    |scores| < ~4), multiplicative 0/1 mask, attn @ vc, and the
    per-q-token denominator partial sums ([128 ct-partitions x 512 q]
    per group, via DVE accumulation) are shipped to the HOST, which
    does y = y_unnorm / den (+ b_out).  No ones-matmuls, no
    reciprocal, no per-partition rescale on device.
  - q columns (and output rows) are conv-phase-permuted; the mask
    columns are permuted identically on the host and the y DMA
    un-permutes with a stride-4 row access pattern.
"""

import math
import os
from contextlib import ExitStack

import ml_dtypes
import numpy as np

BF = ml_dtypes.bfloat16

_NOCC = os.environ.get("NOCC", "0") == "1"   # debug: skip collectives

B, S, E, D, CF = 4, 4096, 1024, 1024, 4
SC = S // CF            # 1024 compressed tokens per batch
SQ = S // 2             # 2048 q rows per core
HTOK = S // 2           # 2048 k/v tokens per core
SCH = SC // 2           # 512 compressed tokens computed per core
P = 128
NCORES = 8
ET = E // P             # 8 contraction tiles for E
FT = D // P             # 8 feature tiles
CT = SC // P            # 8 compressed-token tiles
CTH = CT // 2           # 4 compressed-token tiles per half
NG = 4                  # q groups of 512 in phase D (= conv phases)
MQ = 256                # masked q columns per group (tokens < SC)
DT = D // P             # 8 dh tiles (contraction of the M fold)

_prog_cache = {}


def _build_program(mask_active, add_bq, add_vbias2):
    import concourse.bacc as bacc
    import concourse.mybir as mybir
    import concourse.tile as tile

    F32 = mybir.dt.float32
    BF16 = mybir.dt.bfloat16
    FP8 = mybir.dt.float8e4

    nc = bacc.Bacc("TRN2", num_devices=NCORES)

    xkvT = nc.dram_tensor("xkvT", [E, HTOK], BF16, kind="ExternalInput")
    wqd = nc.dram_tensor("wqd", [D, E], BF16, kind="ExternalInput")
    WFk = nc.dram_tensor("WFk", [CF * E, D], BF16, kind="ExternalInput")
    WFv = nc.dram_tensor("WFv", [CF * E, D], BF16, kind="ExternalInput")
    bkc = nc.dram_tensor("bkc", [P, FT], F32, kind="ExternalInput")
    maskM = None
    if mask_active:
        maskM = nc.dram_tensor("maskM", [SC, NG * MQ], FP8,
                               kind="ExternalInput")
    bqd = None
    if add_bq:
        bqd = nc.dram_tensor("bqd", [P, DT], BF16, kind="ExternalInput")
    vb2 = None
    if add_vbias2:
        vb2 = nc.dram_tensor("vb2", [P, D], F32, kind="ExternalInput")
    y = nc.dram_tensor("y", [SQ, D], BF16, kind="ExternalOutput")
    den = nc.dram_tensor("den", [NG * P, SCH], F32, kind="ExternalOutput")

    PAIRS = [[0, 1], [2, 3], [4, 5], [6, 7]]
    MX = ET * SCH + (CTH if add_bq else 0)   # M exchange cols (+bconst)

    with tile.TileContext(nc) as tc, ExitStack() as top:
        persist = top.enter_context(tc.tile_pool(name="persist", bufs=1))
        dramp = top.enter_context(tc.tile_pool(name="dramp", bufs=1,
                                               space="DRAM"))
        x_sb = persist.tile([P, ET, HTOK], BF16, tag="x")
        kcT = persist.tile([P, DT, SCH], BF16, tag="kcT")   # own kc half
        Me = persist.tile([P, ET, SC], BF16, tag="Me")      # full M^T
        vcp = persist.tile([P, CT, D], BF16, tag="vcp")     # full vc
        wq_sb = persist.tile([P, DT, E], BF16, tag="wq")
        bkc_sb = persist.tile([P, FT], F32, tag="bkc")
        mk = None
        if mask_active:
            mk = persist.tile([P, CT, NG * MQ], FP8, tag="mk")
        bq_sb = None
        bc_sb = None
        if add_bq:
            bq_sb = persist.tile([P, DT], BF16, tag="bqd")
            bc_sb = persist.tile([P, CT], BF16, tag="bconst")
        vb2_sb = None
        if add_vbias2:
            vb2_sb = persist.tile([P, D], F32, tag="vb2")

        # input loads on the scalar queue: x (paced with phase K), then
        # the fold weights / biases / mask needed by later phases.  The
        # sync queue is left free for the WF weight streams.
        nc.scalar.dma_start(out=bkc_sb, in_=bkc[:])
        if add_bq:
            nc.scalar.dma_start(out=bq_sb, in_=bqd[:])
        if add_vbias2:
            nc.scalar.dma_start(out=vb2_sb, in_=vb2[:])

        # collective bounce buffers
        in_m = dramp.tile([P, MX], BF16, tag="in_m")
        out_m = dramp.tile([2, P, MX], BF16, tag="out_m")
        in_vc = dramp.tile([P, CTH * D], BF16, tag="in_vc")
        out_vc = dramp.tile([2, P, CTH * D], BF16, tag="out_vc")

        # tiny warm-up collective: absorbs the one-time collective
        # channel setup / initial rank-skew sync during phase K, so the
        # real M exchange starts promptly
        warm_sb = persist.tile([P, 1], mybir.dt.float32, tag="warm")
        in_w = dramp.tile([P, 1], mybir.dt.float32, tag="in_w")
        out_w = dramp.tile([2, P, 1], mybir.dt.float32, tag="out_w")
        nc.vector.memset(warm_sb, 1.0)
        nc.gpsimd.dma_start(out=in_w, in_=warm_sb)
        if not _NOCC:
            nc.gpsimd.collective_compute(
                "AllGather", mybir.AluOpType.bypass,
                replica_groups=PAIRS,
                ins=[in_w[:].opt()], outs=[out_w[:].opt()])

        # ---------------- phase K: folded project+compress k half ------
        with ExitStack() as ph:
            ws = ph.enter_context(tc.tile_pool(name="swk", bufs=16))
            pp = ph.enter_context(
                tc.tile_pool(name="pk", bufs=8, space="PSUM"))
            pcs = [pp.tile([P, SCH], F32, tag="mm", name=f"pc{fo}")
                   for fo in range(FT)]
            for cet in range(CF * ET):
                c, et = divmod(cet, ET)
                w_sl = ws.tile([P, D], BF16, tag="ws")
                nc.sync.dma_start(
                    out=w_sl, in_=WFk[cet * P:(cet + 1) * P, :])
                # x chunk for this step rides the same queue right
                # behind its weight slice: self-paced, no HBM burst
                nc.sync.dma_start(
                    out=x_sb[:, et, c * SCH:(c + 1) * SCH],
                    in_=xkvT[et * P:(et + 1) * P, c * SCH:(c + 1) * SCH])
                rhs = x_sb[:, et, c * SCH:(c + 1) * SCH]
                for fo in range(FT):
                    nc.tensor.matmul(
                        pcs[fo],
                        w_sl[:, fo * P:(fo + 1) * P],
                        rhs,
                        start=(cet == 0),
                        stop=(cet == CF * ET - 1))
            for fo in range(FT):
                nc.vector.tensor_scalar_add(
                    out=kcT[:, fo, :],
                    in0=pcs[fo], scalar1=bkc_sb[:, fo:fo + 1])

        # ---------------- phase M: M^T = (wq s).T @ kc_own.T -----------
        # q weights stream on the sync queue AFTER phase K's WFk stream
        # (keeps them out of the oversubscribed early HBM window)
        for dt in range(DT):
            nc.sync.dma_start(
                out=wq_sb[:, dt, :], in_=wqd[dt * P:(dt + 1) * P, :])
        with ExitStack() as ph:
            lc = ph.enter_context(tc.tile_pool(name="lm", bufs=1))
            pp = ph.enter_context(
                tc.tile_pool(name="pm", bufs=8, space="PSUM"))
            mexc = lc.tile([P, MX], BF16, tag="mexc")
            pme = [pp.tile([P, SCH], F32, tag="mm", name=f"pm{et}")
                   for et in range(ET)]
            for dt in range(DT):
                for et in range(ET):
                    nc.tensor.matmul(
                        pme[et],
                        wq_sb[:, dt, et * P:(et + 1) * P],
                        kcT[:, dt, :],
                        start=(dt == 0), stop=(dt == DT - 1))
            for et in range(ET):
                nc.vector.tensor_copy(
                    out=mexc[:, et * SCH:(et + 1) * SCH], in_=pme[et])
            if add_bq:
                # bconst[j] = sum_d bq[d] kc[j,d]: extra additive score
                # column constant, exchanged alongside M^T
                pb = ph.enter_context(
                    tc.tile_pool(name="pb", bufs=4, space="PSUM"))
                for ctp in range(CTH):
                    pbc = pb.tile([P, 1], F32, tag="mm", name=f"pb{ctp}")
                    for dt in range(DT):
                        nc.tensor.matmul(
                            pbc,
                            kcT[:, dt, ctp * P:(ctp + 1) * P],
                            bq_sb[:, dt:dt + 1],
                            start=(dt == 0), stop=(dt == DT - 1))
                    nc.vector.tensor_copy(
                        out=mexc[:, ET * SCH + ctp:ET * SCH + ctp + 1],
                        in_=pbc)
            nc.gpsimd.dma_start(out=in_m, in_=mexc)
            if _NOCC:
                nc.gpsimd.dma_start(out=out_m[0], in_=in_m)
                nc.gpsimd.dma_start(out=out_m[1], in_=in_m)
            else:
                nc.gpsimd.collective_compute(
                    "AllGather", mybir.AluOpType.bypass,
                    replica_groups=PAIRS,
                    ins=[in_m[:].opt()], outs=[out_m[:].opt()])
            for g in range(2):
                nc.gpsimd.dma_start(
                    out=Me[:, :, g * SCH:(g + 1) * SCH],
                    in_=out_m[g, :, 0:ET * SCH])
                if add_bq:
                    nc.gpsimd.dma_start(
                        out=bc_sb[:, g * CTH:(g + 1) * CTH],
                        in_=out_m[g, :, ET * SCH:MX])
            # mask loads queue behind the Me redistribute on gpsimd:
            # they land mid-phase-V, well before phase D needs them,
            # without touching the early HBM window
            if mask_active:
                for ctt in range(CT):
                    nc.gpsimd.dma_start(
                        out=mk[:, ctt, :],
                        in_=maskM[ctt * P:(ctt + 1) * P, :])

        # ---------------- phase V: folded project+compress v half ------
        with ExitStack() as ph:
            lc = ph.enter_context(tc.tile_pool(name="lv", bufs=1))
            ws = ph.enter_context(tc.tile_pool(name="swv", bufs=16))
            pp = ph.enter_context(
                tc.tile_pool(name="pv", bufs=8, space="PSUM"))
            vc_loc = lc.tile([P, CTH, D], BF16, tag="vc_loc")
            pvs = [[pp.tile([P, SCH], F32, tag="mm", name=f"pv{ctp}_{o2s}")
                    for o2s in range(2)] for ctp in range(CTH)]
            for cet in range(CF * ET):
                c, et = divmod(cet, ET)
                w_sl = ws.tile([P, D], BF16, tag="ws")
                nc.sync.dma_start(
                    out=w_sl, in_=WFv[cet * P:(cet + 1) * P, :])
                win = x_sb[:, et, c * SCH:(c + 1) * SCH]
                for ctp in range(CTH):
                    lhsT = win[:, ctp * P:(ctp + 1) * P]
                    for o2s in range(2):
                        nc.tensor.matmul(
                            pvs[ctp][o2s],
                            lhsT,
                            w_sl[:, o2s * 512:(o2s + 1) * 512],
                            start=(cet == 0),
                            stop=(cet == CF * ET - 1))
            for ctp in range(CTH):
                for o2s in range(2):
                    dst = vc_loc[:, ctp, o2s * 512:(o2s + 1) * 512]
                    if add_vbias2:
                        nc.vector.tensor_tensor(
                            out=dst, in0=pvs[ctp][o2s],
                            in1=vb2_sb[:, o2s * 512:(o2s + 1) * 512],
                            op=mybir.AluOpType.add)
                    else:
                        nc.vector.tensor_copy(
                            out=dst, in_=pvs[ctp][o2s])
            nc.gpsimd.dma_start(out=in_vc, in_=vc_loc)
            if _NOCC:
                nc.gpsimd.dma_start(out=out_vc[0], in_=in_vc)
                nc.gpsimd.dma_start(out=out_vc[1], in_=in_vc)
            else:
                nc.gpsimd.collective_compute(
                    "AllGather", mybir.AluOpType.bypass,
                    replica_groups=PAIRS,
                    ins=[in_vc[:].opt()], outs=[out_vc[:].opt()])
            for g in range(2):
                nc.gpsimd.dma_start(
                    out=vcp[:, g * CTH:(g + 1) * CTH, :],
                    in_=out_vc[g])

        # ---------------- phase D: attention ----------------
        with ExitStack() as ph:
            att = ph.enter_context(tc.tile_pool(name="att", bufs=4))
            dnp = ph.enter_context(tc.tile_pool(name="dnp", bufs=3))
            yp = ph.enter_context(tc.tile_pool(name="yp", bufs=3))
            pS = ph.enter_context(
                tc.tile_pool(name="pS", bufs=2, space="PSUM"))
            pO = ph.enter_context(
                tc.tile_pool(name="pO", bufs=2, space="PSUM"))

            ats = [None] * NG

            def scores_group(g):
                at = att.tile([P, CT, 512], BF16, tag="at")
                ats[g] = at
                den_t = dnp.tile([P, SCH], F32, tag="den")
                for ctt in range(CT):
                    sc = pS.tile([P, 512], F32, tag="sc")
                    for et in range(ET):
                        nc.tensor.matmul(
                            sc,
                            Me[:, et, ctt * P:(ctt + 1) * P],
                            x_sb[:, et, g * 512:(g + 1) * 512],
                            start=(et == 0), stop=(et == ET - 1))
                    if add_bq:
                        nc.scalar.activation(
                            out=at[:, ctt, :], in_=sc,
                            func=mybir.ActivationFunctionType.Exp,
                            bias=bc_sb[:, ctt:ctt + 1])
                    else:
                        nc.scalar.activation(
                            out=at[:, ctt, :], in_=sc,
                            func=mybir.ActivationFunctionType.Exp)
                    if mask_active:
                        # columns 0..255 of every group hold the
                        # tokens < SC (the tril-masked rows)
                        nc.vector.tensor_tensor(
                            out=at[:, ctt, 0:MQ], in0=at[:, ctt, 0:MQ],
                            in1=mk[:, ctt, g * MQ:(g + 1) * MQ],
                            op=mybir.AluOpType.mult)
                    if ctt == 0:
                        nc.vector.tensor_copy(out=den_t, in_=at[:, 0, :])
                    else:
                        nc.vector.tensor_tensor(
                            out=den_t, in0=den_t, in1=at[:, ctt, :],
                            op=mybir.AluOpType.add)
                nc.sync.dma_start(
                    out=den[g * P:(g + 1) * P, :], in_=den_t)

            def attn_group(g):
                at = ats[g]
                for qp in range(4):
                    po = pO.tile([P, D], F32, tag="out")
                    for ctt in range(CT):
                        lhsT = at[:, ctt, qp * P:(qp + 1) * P]
                        nc.tensor.matmul(
                            po[:, 0:512], lhsT, vcp[:, ctt, 0:512],
                            start=(ctt == 0), stop=(ctt == CT - 1))
                        nc.tensor.matmul(
                            po[:, 512:1024], lhsT, vcp[:, ctt, 512:1024],
                            start=(ctt == 0), stop=(ctt == CT - 1))
                    y_sb = yp.tile([P, D], BF16, tag="y")
                    # split cast+store so the first half's DMA overlaps
                    # the second half's cast (shrinks the kernel tail)
                    nc.vector.tensor_copy(
                        out=y_sb[:, 0:512], in_=po[:, 0:512])
                    nc.vector.tensor_copy(
                        out=y_sb[:, 512:1024], in_=po[:, 512:1024])
                    # un-permute: partition i holds token 512*qp+4*i+g
                    r0 = 512 * qp + g
                    nc.sync.dma_start(
                        out=y[r0:r0 + 509:4, 0:512], in_=y_sb[:, 0:512])
                    nc.sync.dma_start(
                        out=y[r0:r0 + 509:4, 512:1024],
                        in_=y_sb[:, 512:1024])

            # vc exchange overlaps the first two score groups
            scores_group(0)
            scores_group(1)
            scores_group(2)
            attn_group(0)
            scores_group(3)
            attn_group(1)
            attn_group(2)
            attn_group(3)

    nc.compile()
    return nc


def _get_program(mask_active, add_bq, add_vbias2):
    key = (mask_active, add_bq, add_vbias2)
    if key not in _prog_cache:
        _prog_cache[key] = _build_program(*key)
    return _prog_cache[key]


def prepare(x, w_qkv, b_qkv, wk_conv, bk_conv, wv_conv, bv_conv, w_out, b_out,
            mask):
    """Host-side prep: returns (nc, in_maps, ctx) for run_bass_kernel_spmd."""
    x = np.ascontiguousarray(np.asarray(x, np.float32))
    w_qkv = np.asarray(w_qkv, np.float32)
    b_qkv = np.asarray(b_qkv, np.float32)
    wk_conv = np.asarray(wk_conv, np.float32)
    bk_conv = np.asarray(bk_conv, np.float32)
    wv_conv = np.asarray(wv_conv, np.float32)
    bv_conv = np.asarray(bv_conv, np.float32)
    w_out = np.asarray(w_out, np.float32)
    b_out = np.asarray(b_out, np.float32)
    mask_active = bool(np.asarray(mask).reshape(-1)[0])

    scale = 1.0 / math.sqrt(D)
    wT = np.ascontiguousarray(w_qkv.T)                 # [E, 3D]
    wqd = np.ascontiguousarray((w_qkv[0:D, :] * scale).astype(BF))  # [D,E]
    wkm = wT[:, D:2 * D]                               # [E, D]
    wvm = wT[:, 2 * D:3 * D]
    bkv = b_qkv[D:2 * D]
    bvv = b_qkv[2 * D:3 * D]
    bqv = b_qkv[0:D] * scale
    # W2[cd, o] with cd = c*D + d  <-  w_conv[o, d, c]
    W2kT = np.ascontiguousarray(wk_conv.transpose(2, 1, 0).reshape(CF * D, D))
    W2vT = np.ascontiguousarray(wv_conv.transpose(2, 1, 0).reshape(CF * D, D))
    W3 = W2vT @ w_out.T                                # fold out-proj into v
    # fold the k/v projections into the compress GEMMs:
    #   WF[(c,e), o] = sum_d wproj[e, d] * W2[(c,d), o]
    WFk = np.ascontiguousarray(
        (wkm @ W2kT.reshape(CF, D, D)).reshape(CF * E, D).astype(BF))
    WFv = np.ascontiguousarray(
        (wvm @ W3.reshape(CF, D, D)).reshape(CF * E, D).astype(BF))
    # projection biases flow through the conv contraction
    bkc2 = bk_conv + W2kT.reshape(CF, D, D).sum(0).T @ bkv
    b_vc2 = w_out @ bv_conv + W3.reshape(CF, D, D).sum(0).T @ bvv
    bkc = np.ascontiguousarray(bkc2.reshape(FT, P).T)
    add_vbias2 = bool(np.any(b_vc2))
    add_bq = bool(np.any(bqv))

    nc = _get_program(mask_active, add_bq, add_vbias2)

    if mask_active:
        # column-permuted tril mask: group g, col s  <->  token 4s+g
        ct_idx = np.arange(SC)[:, None]
        cols = np.concatenate(
            [4 * np.arange(MQ) + g for g in range(NG)])    # [NG*MQ]
        mm_real = np.ascontiguousarray(
            (ct_idx <= cols[None, :]).astype(ml_dtypes.float8_e4m3fn))
        mm_ones = np.ones((SC, NG * MQ), ml_dtypes.float8_e4m3fn)

    in_maps = []
    for core in range(NCORES):
        b, h = divmod(core, 2)
        xh = x[b].T[:, h * HTOK:(h + 1) * HTOK]
        # de-interleave the conv windows: col c*512+s holds token 4s+c
        xkv = np.ascontiguousarray(
            xh.reshape(E, HTOK // CF, CF).transpose(0, 2, 1)
            .reshape(E, HTOK).astype(BF))
        m = {
            "xkvT": xkv,
            "wqd": wqd, "WFk": WFk, "WFv": WFv,
            "bkc": bkc,
        }
        if mask_active:
            m["maskM"] = mm_real if h == 0 else mm_ones
        if add_bq:
            m["bqd"] = np.ascontiguousarray(
                bqv.reshape(DT, P).T.astype(BF))
        if add_vbias2:
            m["vb2"] = np.ascontiguousarray(
                np.broadcast_to(b_vc2[None, :], (P, D)).astype(np.float32))
        in_maps.append(m)
    return nc, in_maps, b_out


def assemble(results, b_out):
    out = np.empty((B, S, D), np.float32)
    t = np.arange(SQ)
    gi, si = t % CF, t // CF
    badd = b_out[None, :] if np.any(b_out) else None
    for core in range(NCORES):
        b, h = divmod(core, 2)
        yv = np.asarray(results[core]["y"]).astype(np.float32)
        dv = np.asarray(results[core]["den"])       # [NG*P, SCH] f32
        den_g = dv.reshape(NG, P, SCH).sum(axis=1)  # [NG, SCH]
        dloc = den_g[gi, si]                        # [SQ]
        yv /= dloc[:, None]
        if badd is not None:
            yv += badd
        out[b, h * SQ:(h + 1) * SQ, :] = yv
    return out


def kernel(x, w_qkv, b_qkv, wk_conv, bk_conv, wv_conv, bv_conv, w_out, b_out,
           mask):
    from concourse.bass_utils import run_bass_kernel_spmd

    nc, in_maps, b_out = prepare(x, w_qkv, b_qkv, wk_conv, bk_conv, wv_conv,
                                 bv_conv, w_out, b_out, mask)
    res = run_bass_kernel_spmd(nc, in_maps, core_ids=list(range(NCORES)))
    return assemble(res.results, b_out)


# revision 13
# speedup vs baseline: 1.4274x; 1.4274x over previous
"""Compressed multi-head attention (H=1) TRN2 Bass kernel, v5.

Reference computation (B=4, S=4096, E=D=1024, H=1, CF=4, Sc=1024):
    qkv = x @ w_qkv.T + b_qkv ; q,k,v = split(qkv)
    kc  = conv1d_stride4(k) + bk ; vc = conv1d_stride4(v) + bv      # [B,Sc,D]
    scores = q @ kc.T / sqrt(D)   (+ causal tril(S,Sc) mask)
    attn = softmax(scores); out = attn @ vc
    y = out @ w_out.T + b_out                                        # [B,S,D]

Sharding: 8 cores = 4 batches x 2 token-halves.  Each core compresses
k/v for ITS half of the batch tokens only (512 compressed tokens).

Device algebra (all operands bf16, PSUM fp32):
  - k/v projections FOLDED into the compress conv on the host (one
    GEMM per path, contraction CF*E = 4096 over de-interleaved x).
  - out-projection pre-folded into the v weights on the host.
  - THE Q PROJECTION IS FOLDED INTO kc: scores = x @ M.T with
    M.T = (wq s).T @ kc.T computed ON DEVICE over the core's local
    512 kc columns (64 matmuls vs 256 for a full q projection); the
    M.T halves are exchanged pairwise (1MB bf16 AllGather), which
    also replaces the kc exchange entirely.
  - phase order K -> M -> V -> D: the M AllGather overlaps phase V,
    the vc AllGather overlaps the first score groups of phase D.
  - softmax is unnormalized on device: exp(scores), 0/1 mask,
    attn @ vc, and per-q-token denominator partials (DVE-accumulated
    [128 x 512] per group) ship to the HOST which divides (+ b_out).
  - ONE psum ring pool (8 banks, one tag) spans all phases so phase
    transitions hand off PSUM bank-by-bank instead of pool-at-once;
    psum->SBUF exports alternate vector/scalar engines to halve the
    handoff latency.
  - x chunks ride the sync queue interleaved with the WFk stream
    (self-paced; no early HBM burst), weight streams are 16-deep
    buffered so the collectives' DMA traffic cannot starve the PE.
  - q columns (and output rows) are conv-phase-permuted; the mask
    columns are permuted identically on the host and the y DMA
    un-permutes with a stride-4 row access pattern.
"""

import math
import os
from contextlib import ExitStack

import ml_dtypes
import numpy as np

BF = ml_dtypes.bfloat16

_NOCC = os.environ.get("NOCC", "0") == "1"   # debug: skip collectives

B, S, E, D, CF = 4, 4096, 1024, 1024, 4
SC = S // CF            # 1024 compressed tokens per batch
SQ = S // 2             # 2048 q rows per core
HTOK = S // 2           # 2048 k/v tokens per core
SCH = SC // 2           # 512 compressed tokens computed per core
P = 128
NCORES = 8
ET = E // P             # 8 contraction tiles for E
FT = D // P             # 8 feature tiles
CT = SC // P            # 8 compressed-token tiles
CTH = CT // 2           # 4 compressed-token tiles per half
NG = 4                  # q groups of 512 in phase D (= conv phases)
MQ = 256                # masked q columns per group (tokens < SC)
DT = D // P             # 8 dh tiles (contraction of the M fold)

_prog_cache = {}


def _build_program(mask_active, add_bq, add_bkc, add_vbias2):
    import concourse.bacc as bacc
    import concourse.mybir as mybir
    import concourse.tile as tile

    F32 = mybir.dt.float32
    BF16 = mybir.dt.bfloat16
    FP8 = mybir.dt.float8e4

    nc = bacc.Bacc("TRN2", num_devices=NCORES)

    xkvT = nc.dram_tensor("xkvT", [E, HTOK], BF16, kind="ExternalInput")
    wqd = nc.dram_tensor("wqd", [D, E], BF16, kind="ExternalInput")
    WFk = nc.dram_tensor("WFk", [CF * E, D], BF16, kind="ExternalInput")
    WFv = nc.dram_tensor("WFv", [CF * E, D], BF16, kind="ExternalInput")
    bkc = None
    if add_bkc:
        bkc = nc.dram_tensor("bkc", [P, FT], F32, kind="ExternalInput")
    maskM = None
    if mask_active:
        maskM = nc.dram_tensor("maskM", [SC, NG * MQ], FP8,
                               kind="ExternalInput")
    bqd = None
    if add_bq:
        bqd = nc.dram_tensor("bqd", [P, DT], BF16, kind="ExternalInput")
    vb2 = None
    if add_vbias2:
        vb2 = nc.dram_tensor("vb2", [P, D], F32, kind="ExternalInput")
    y = nc.dram_tensor("y", [SQ, D], BF16, kind="ExternalOutput")
    den = nc.dram_tensor("den", [NG * P, SCH], F32, kind="ExternalOutput")

    PAIRS = [[0, 1], [2, 3], [4, 5], [6, 7]]
    MX = ET * SCH + (CTH if add_bq else 0)   # M exchange cols (+bconst)

    with tile.TileContext(nc) as tc, ExitStack() as top:
        persist = top.enter_context(tc.tile_pool(name="persist", bufs=1))
        dramp = top.enter_context(tc.tile_pool(name="dramp", bufs=1,
                                               space="DRAM"))
        # single psum ring: every [P,512] fp32 accumulator in the kernel
        # rotates through the same 8 banks -> bank-granular handoff at
        # every phase boundary
        pmm = top.enter_context(tc.tile_pool(name="pmm", bufs=8,
                                             space="PSUM"))

        x_sb = persist.tile([P, ET, HTOK], BF16, tag="x")
        kcT = persist.tile([P, DT, SCH], BF16, tag="kcT")   # own kc half
        Me = persist.tile([P, ET, SC], BF16, tag="Me")      # full M^T
        vcp = persist.tile([P, CT, D], BF16, tag="vcp")     # full vc
        wq_sb = persist.tile([P, DT, E], BF16, tag="wq")
        bkc_sb = None
        if add_bkc:
            bkc_sb = persist.tile([P, FT], F32, tag="bkc")
        mk = None
        if mask_active:
            mk = persist.tile([P, CT, NG * MQ], FP8, tag="mk")
        bq_sb = None
        bc_sb = None
        if add_bq:
            bq_sb = persist.tile([P, DT], BF16, tag="bqd")
            bc_sb = persist.tile([P, CT], BF16, tag="bconst")
        vb2_sb = None
        if add_vbias2:
            vb2_sb = persist.tile([P, D], F32, tag="vb2")

        # q weights + small inputs on the scalar queue (x itself is
        # interleaved with the WFk stream on the sync queue below)
        for dt in range(DT):
            nc.scalar.dma_start(
                out=wq_sb[:, dt, :], in_=wqd[dt * P:(dt + 1) * P, :])
        if add_bkc:
            nc.scalar.dma_start(out=bkc_sb, in_=bkc[:])
        if add_bq:
            nc.scalar.dma_start(out=bq_sb, in_=bqd[:])
        if add_vbias2:
            nc.scalar.dma_start(out=vb2_sb, in_=vb2[:])

        # collective bounce buffers
        in_m = dramp.tile([P, MX], BF16, tag="in_m")
        out_m = dramp.tile([2, P, MX], BF16, tag="out_m")
        in_vc = dramp.tile([P, CTH * D], BF16, tag="in_vc")
        out_vc = dramp.tile([2, P, CTH * D], BF16, tag="out_vc")

        def copy_out(i, out_ap, in_ap):
            # alternate export engines so psum banks free ~2x faster
            if i % 2:
                nc.scalar.copy(out=out_ap, in_=in_ap)
            else:
                nc.vector.tensor_copy(out=out_ap, in_=in_ap)

        # ---------------- phase K: folded project+compress k half ------
        with ExitStack() as ph:
            ws = ph.enter_context(tc.tile_pool(name="swk", bufs=16))
            pcs = [pmm.tile([P, SCH], F32, tag="mm", name=f"pc{fo}")
                   for fo in range(FT)]
            for cet in range(CF * ET):
                c, et = divmod(cet, ET)
                w_sl = ws.tile([P, D], BF16, tag="ws")
                nc.sync.dma_start(
                    out=w_sl, in_=WFk[cet * P:(cet + 1) * P, :])
                # x chunk rides the same queue right behind its weight
                # slice: self-paced, no early HBM burst
                nc.sync.dma_start(
                    out=x_sb[:, et, c * SCH:(c + 1) * SCH],
                    in_=xkvT[et * P:(et + 1) * P, c * SCH:(c + 1) * SCH])
                rhs = x_sb[:, et, c * SCH:(c + 1) * SCH]
                for fo in range(FT):
                    nc.tensor.matmul(
                        pcs[fo],
                        w_sl[:, fo * P:(fo + 1) * P],
                        rhs,
                        start=(cet == 0),
                        stop=(cet == CF * ET - 1))
            for fo in range(FT):
                if add_bkc:
                    nc.vector.tensor_scalar_add(
                        out=kcT[:, fo, :],
                        in0=pcs[fo], scalar1=bkc_sb[:, fo:fo + 1])
                else:
                    copy_out(fo, kcT[:, fo, :], pcs[fo])

        # ---------------- phase M: M^T = (wq s).T @ kc_own.T -----------
        with ExitStack() as ph:
            lc = ph.enter_context(tc.tile_pool(name="lm", bufs=1))
            mexc = lc.tile([P, MX], BF16, tag="mexc")
            pme = [pmm.tile([P, SCH], F32, tag="mm", name=f"pm{et}")
                   for et in range(ET)]
            for dt in range(DT):
                for et in range(ET):
                    nc.tensor.matmul(
                        pme[et],
                        wq_sb[:, dt, et * P:(et + 1) * P],
                        kcT[:, dt, :],
                        start=(dt == 0), stop=(dt == DT - 1))
            for et in range(ET):
                copy_out(et, mexc[:, et * SCH:(et + 1) * SCH], pme[et])
            if add_bq:
                # bconst[j] = sum_d bq[d] kc[j,d]: additive score
                # constant per kc row, exchanged alongside M^T
                for ctp in range(CTH):
                    pbc = pmm.tile([P, 1], F32, tag="mm", name=f"pb{ctp}")
                    for dt in range(DT):
                        nc.tensor.matmul(
                            pbc,
                            kcT[:, dt, ctp * P:(ctp + 1) * P],
                            bq_sb[:, dt:dt + 1],
                            start=(dt == 0), stop=(dt == DT - 1))
                    nc.vector.tensor_copy(
                        out=mexc[:, ET * SCH + ctp:ET * SCH + ctp + 1],
                        in_=pbc)
            nc.gpsimd.dma_start(out=in_m, in_=mexc)
            if _NOCC:
                nc.gpsimd.dma_start(out=out_m[0], in_=in_m)
                nc.gpsimd.dma_start(out=out_m[1], in_=in_m)
            else:
                nc.gpsimd.collective_compute(
                    "AllGather", mybir.AluOpType.bypass,
                    replica_groups=PAIRS,
                    ins=[in_m[:].opt()], outs=[out_m[:].opt()])
            for g in range(2):
                nc.gpsimd.dma_start(
                    out=Me[:, :, g * SCH:(g + 1) * SCH],
                    in_=out_m[g, :, 0:ET * SCH])
                if add_bq:
                    nc.gpsimd.dma_start(
                        out=bc_sb[:, g * CTH:(g + 1) * CTH],
                        in_=out_m[g, :, ET * SCH:MX])

        # ---------------- phase V: folded project+compress v half ------
        with ExitStack() as ph:
            lc = ph.enter_context(tc.tile_pool(name="lv", bufs=1))
            ws = ph.enter_context(tc.tile_pool(name="swv", bufs=16))
            vc_loc = lc.tile([P, CTH, D], BF16, tag="vc_loc")
            pvs = [[pmm.tile([P, SCH], F32, tag="mm", name=f"pv{ctp}_{o2s}")
                    for o2s in range(2)] for ctp in range(CTH)]
            for cet in range(CF * ET):
                c, et = divmod(cet, ET)
                w_sl = ws.tile([P, D], BF16, tag="ws")
                nc.sync.dma_start(
                    out=w_sl, in_=WFv[cet * P:(cet + 1) * P, :])
                win = x_sb[:, et, c * SCH:(c + 1) * SCH]
                for ctp in range(CTH):
                    lhsT = win[:, ctp * P:(ctp + 1) * P]
                    for o2s in range(2):
                        nc.tensor.matmul(
                            pvs[ctp][o2s],
                            lhsT,
                            w_sl[:, o2s * 512:(o2s + 1) * 512],
                            start=(cet == 0),
                            stop=(cet == CF * ET - 1))
            i = 0
            for ctp in range(CTH):
                for o2s in range(2):
                    dst = vc_loc[:, ctp, o2s * 512:(o2s + 1) * 512]
                    if add_vbias2:
                        nc.vector.tensor_tensor(
                            out=dst, in0=pvs[ctp][o2s],
                            in1=vb2_sb[:, o2s * 512:(o2s + 1) * 512],
                            op=mybir.AluOpType.add)
                    else:
                        copy_out(i, dst, pvs[ctp][o2s])
                    i += 1
            nc.gpsimd.dma_start(out=in_vc, in_=vc_loc)
            if _NOCC:
                nc.gpsimd.dma_start(out=out_vc[0], in_=in_vc)
                nc.gpsimd.dma_start(out=out_vc[1], in_=in_vc)
            else:
                nc.gpsimd.collective_compute(
                    "AllGather", mybir.AluOpType.bypass,
                    replica_groups=PAIRS,
                    ins=[in_vc[:].opt()], outs=[out_vc[:].opt()])
            for g in range(2):
                nc.gpsimd.dma_start(
                    out=vcp[:, g * CTH:(g + 1) * CTH, :],
                    in_=out_vc[g])

        # mask loads: emitted late so the scheduler gives them low
        # priority; phase D only needs them ~170us in
        if mask_active:
            for ctt in range(CT):
                nc.scalar.dma_start(
                    out=mk[:, ctt, :],
                    in_=maskM[ctt * P:(ctt + 1) * P, :])

        # ---------------- phase D: attention ----------------
        with ExitStack() as ph:
            att = ph.enter_context(tc.tile_pool(name="att", bufs=4))
            dnp = ph.enter_context(tc.tile_pool(name="dnp", bufs=3))
            yp = ph.enter_context(tc.tile_pool(name="yp", bufs=3))

            ats = [None] * NG

            def scores_group(g):
                at = att.tile([P, CT, 512], BF16, tag="at")
                ats[g] = at
                den_t = dnp.tile([P, SCH], F32, tag="den")
                for ctt in range(CT):
                    sc = pmm.tile([P, 512], F32, tag="mm")
                    for et in range(ET):
                        nc.tensor.matmul(
                            sc,
                            Me[:, et, ctt * P:(ctt + 1) * P],
                            x_sb[:, et, g * 512:(g + 1) * 512],
                            start=(et == 0), stop=(et == ET - 1))
                    if add_bq:
                        nc.scalar.activation(
                            out=at[:, ctt, :], in_=sc,
                            func=mybir.ActivationFunctionType.Exp,
                            bias=bc_sb[:, ctt:ctt + 1])
                    else:
                        nc.scalar.activation(
                            out=at[:, ctt, :], in_=sc,
                            func=mybir.ActivationFunctionType.Exp)
                    if mask_active:
                        # columns 0..255 of every group hold the
                        # tokens < SC (the tril-masked rows)
                        nc.vector.tensor_tensor(
                            out=at[:, ctt, 0:MQ], in0=at[:, ctt, 0:MQ],
                            in1=mk[:, ctt, g * MQ:(g + 1) * MQ],
                            op=mybir.AluOpType.mult)
                    if ctt == 0:
                        nc.vector.tensor_copy(out=den_t, in_=at[:, 0, :])
                    else:
                        nc.vector.tensor_tensor(
                            out=den_t, in0=den_t, in1=at[:, ctt, :],
                            op=mybir.AluOpType.add)
                nc.sync.dma_start(
                    out=den[g * P:(g + 1) * P, :], in_=den_t)

            def attn_group(g):
                at = ats[g]
                for qp in range(4):
                    po_a = pmm.tile([P, 512], F32, tag="mm")
                    po_b = pmm.tile([P, 512], F32, tag="mm")
                    for ctt in range(CT):
                        lhsT = at[:, ctt, qp * P:(qp + 1) * P]
                        nc.tensor.matmul(
                            po_a, lhsT, vcp[:, ctt, 0:512],
                            start=(ctt == 0), stop=(ctt == CT - 1))
                        nc.tensor.matmul(
                            po_b, lhsT, vcp[:, ctt, 512:1024],
                            start=(ctt == 0), stop=(ctt == CT - 1))
                    y_sb = yp.tile([P, D], BF16, tag="y")
                    nc.vector.tensor_copy(out=y_sb[:, 0:512], in_=po_a)
                    nc.scalar.copy(out=y_sb[:, 512:1024], in_=po_b)
                    # un-permute: partition i holds token 512*qp+4*i+g
                    r0 = 512 * qp + g
                    nc.sync.dma_start(
                        out=y[r0:r0 + 509:4, 0:512], in_=y_sb[:, 0:512])
                    nc.sync.dma_start(
                        out=y[r0:r0 + 509:4, 512:1024],
                        in_=y_sb[:, 512:1024])

            # vc exchange overlaps the first score groups
            scores_group(0)
            scores_group(1)
            scores_group(2)
            attn_group(0)
            scores_group(3)
            attn_group(1)
            attn_group(2)
            attn_group(3)

    nc.compile()
    return nc


def _get_program(mask_active, add_bq, add_bkc, add_vbias2):
    key = (mask_active, add_bq, add_bkc, add_vbias2)
    if key not in _prog_cache:
        _prog_cache[key] = _build_program(*key)
    return _prog_cache[key]


def prepare(x, w_qkv, b_qkv, wk_conv, bk_conv, wv_conv, bv_conv, w_out, b_out,
            mask):
    """Host-side prep: returns (nc, in_maps, b_out) for run_bass_kernel_spmd."""
    x = np.ascontiguousarray(np.asarray(x, np.float32))
    w_qkv = np.asarray(w_qkv, np.float32)
    b_qkv = np.asarray(b_qkv, np.float32)
    wk_conv = np.asarray(wk_conv, np.float32)
    bk_conv = np.asarray(bk_conv, np.float32)
    wv_conv = np.asarray(wv_conv, np.float32)
    bv_conv = np.asarray(bv_conv, np.float32)
    w_out = np.asarray(w_out, np.float32)
    b_out = np.asarray(b_out, np.float32)
    mask_active = bool(np.asarray(mask).reshape(-1)[0])

    scale = 1.0 / math.sqrt(D)
    wT = np.ascontiguousarray(w_qkv.T)                 # [E, 3D]
    wqd = np.ascontiguousarray((w_qkv[0:D, :] * scale).astype(BF))  # [D,E]
    wkm = wT[:, D:2 * D]                               # [E, D]
    wvm = wT[:, 2 * D:3 * D]
    bkv = b_qkv[D:2 * D]
    bvv = b_qkv[2 * D:3 * D]
    bqv = b_qkv[0:D] * scale
    # W2[cd, o] with cd = c*D + d  <-  w_conv[o, d, c]
    W2kT = np.ascontiguousarray(wk_conv.transpose(2, 1, 0).reshape(CF * D, D))
    W2vT = np.ascontiguousarray(wv_conv.transpose(2, 1, 0).reshape(CF * D, D))
    W3 = W2vT @ w_out.T                                # fold out-proj into v
    # fold the k/v projections into the compress GEMMs:
    #   WF[(c,e), o] = sum_d wproj[e, d] * W2[(c,d), o]
    WFk = np.ascontiguousarray(
        (wkm @ W2kT.reshape(CF, D, D)).reshape(CF * E, D).astype(BF))
    WFv = np.ascontiguousarray(
        (wvm @ W3.reshape(CF, D, D)).reshape(CF * E, D).astype(BF))
    # projection biases flow through the conv contraction
    bkc2 = bk_conv + W2kT.reshape(CF, D, D).sum(0).T @ bkv
    b_vc2 = w_out @ bv_conv + W3.reshape(CF, D, D).sum(0).T @ bvv
    add_bkc = bool(np.any(bkc2))
    add_vbias2 = bool(np.any(b_vc2))
    add_bq = bool(np.any(bqv))

    nc = _get_program(mask_active, add_bq, add_bkc, add_vbias2)

    if mask_active:
        # column-permuted tril mask: group g, col s  <->  token 4s+g
        ct_idx = np.arange(SC)[:, None]
        cols = np.concatenate(
            [4 * np.arange(MQ) + g for g in range(NG)])    # [NG*MQ]
        mm_real = np.ascontiguousarray(
            (ct_idx <= cols[None, :]).astype(ml_dtypes.float8_e4m3fn))
        mm_ones = np.ones((SC, NG * MQ), ml_dtypes.float8_e4m3fn)

    in_maps = []
    for core in range(NCORES):
        b, h = divmod(core, 2)
        xh = x[b].T[:, h * HTOK:(h + 1) * HTOK]
        # de-interleave the conv windows: col c*512+s holds token 4s+c
        xkv = np.ascontiguousarray(
            xh.reshape(E, HTOK // CF, CF).transpose(0, 2, 1)
            .reshape(E, HTOK).astype(BF))
        m = {
            "xkvT": xkv,
            "wqd": wqd, "WFk": WFk, "WFv": WFv,
        }
        if add_bkc:
            m["bkc"] = np.ascontiguousarray(bkc2.reshape(FT, P).T)
        if mask_active:
            m["maskM"] = mm_real if h == 0 else mm_ones
        if add_bq:
            m["bqd"] = np.ascontiguousarray(
                bqv.reshape(DT, P).T.astype(BF))
        if add_vbias2:
            m["vb2"] = np.ascontiguousarray(
                np.broadcast_to(b_vc2[None, :], (P, D)).astype(np.float32))
        in_maps.append(m)
    return nc, in_maps, b_out


def assemble(results, b_out):
    out = np.empty((B, S, D), np.float32)
    t = np.arange(SQ)
    gi, si = t % CF, t // CF
    badd = b_out[None, :] if np.any(b_out) else None
    for core in range(NCORES):
        b, h = divmod(core, 2)
        yv = np.asarray(results[core]["y"]).astype(np.float32)
        dv = np.asarray(results[core]["den"])       # [NG*P, SCH] f32
        den_g = dv.reshape(NG, P, SCH).sum(axis=1)  # [NG, SCH]
        dloc = den_g[gi, si]                        # [SQ]
        yv /= dloc[:, None]
        if badd is not None:
            yv += badd
        out[b, h * SQ:(h + 1) * SQ, :] = yv
    return out


def kernel(x, w_qkv, b_qkv, wk_conv, bk_conv, wv_conv, bv_conv, w_out, b_out,
           mask):
    from concourse.bass_utils import run_bass_kernel_spmd

    nc, in_maps, b_out = prepare(x, w_qkv, b_qkv, wk_conv, bk_conv, wv_conv,
                                 bv_conv, w_out, b_out, mask)
    res = run_bass_kernel_spmd(nc, in_maps, core_ids=list(range(NCORES)))
    return assemble(res.results, b_out)
